# revision 15
# baseline (speedup 1.0000x reference)
"""Trainium2 Bass kernel for nn_NeuronCircuit_86784109183091 (moe_routing).

Reference computation (per batch b):
    h_qk = x[b] @ Fqk[idx_qk[b]].T            [S, 64]
    h_v  = x[b] @ Fv[idx_v[b]].T              [S, 32]
    Q    = h_qk @ Rq[idx_q[b]]                [S, D]
    K    = h_qk @ Rk[idx_k[b]]                [S, D]
    V    = h_v  @ Vn[idx_v2[b]]               [S, D]
    out  = causal_mha(Q, K, V; H=16, dh=64) @ W_O.T

Sharding: 8 cores = 4 batches x 2 head-groups (8 heads each). Gathers and
W_O column slicing happen host-side (index tensors are tiny); each core runs
a dense kernel and returns a partial transposed output [D, S]; the host sums
the two head-group partials per batch and transposes.

On-device layout (per core):
    xT   [D, S]    h computed transposed:  hT = FT.T @ xT        [64/32, S]
    QT/KT [512,S]  as 4 tiles [128, S]; tile t holds heads 2t / 2t+1
    V    [S, 512]  s-major, as [128, 16*8*64] (s-chunk, head, dh)
    scoresT[k, q]  per 128-k-chunk, 512-q-tile; exp on ScalarE; causal via
                   block skipping + precomputed 0/1 masks on diagonal chunks
    att_outT       unnormalized via col-packed AV matmuls; denominators via
                   ones-vector matmuls; normalized with reciprocal +
                   gpsimd partition_broadcast + one DVE multiply
    out.T += WO_g slice.T @ att_outT  (accumulated over 4 dim-chunks)

All matmuls run in float32r (full PE rate at N>=512; ~1e-4 matmul rel err).
"""

import sys

for _p in ("/opt/trn_rl_repo",):
    if _p not in sys.path:
        sys.path.append(_p)

import numpy as np

import concourse.bacc as bacc
import concourse.mybir as mybir
import concourse.tile as tile

F32 = mybir.dt.float32
F32R = mybir.dt.float32r

B = 4
S = 2048
D = 1024
NG = 2          # head groups (cores per batch)
HPG = 8         # heads per group
DH = 64
GD = NG and 512  # dims per group
N_FQK = 64      # gathered feature_qk rows
N_FV = 32       # gathered feature_v rows
SCALE = 1.0 / 8.0

NKCH = S // 128   # 16 k-chunks of 128
NQT = S // 512    # 4 q-tiles of 512
NDCH = D // 128   # 8 d-chunks of 128


def _build_program():
    nc = bacc.Bacc("TRN2", target_bir_lowering=False, debug=False)

    XT = nc.dram_tensor("xt", [D, S], F32R, kind="ExternalInput")
    FQKT = nc.dram_tensor("fqkt", [D, N_FQK], F32R, kind="ExternalInput")
    FVT = nc.dram_tensor("fvt", [D, N_FV], F32R, kind="ExternalInput")
    RQ = nc.dram_tensor("rq", [N_FQK, GD], F32R, kind="ExternalInput")
    RK = nc.dram_tensor("rk", [N_FQK, GD], F32R, kind="ExternalInput")
    VN = nc.dram_tensor("vn", [N_FV, GD], F32R, kind="ExternalInput")
    WO = nc.dram_tensor("wo", [GD, D], F32R, kind="ExternalInput")
    OUT = nc.dram_tensor("out", [D, S], F32, kind="ExternalOutput")

    with tile.TileContext(nc) as tc:
        _emit(nc, tc, XT, FQKT, FVT, RQ, RK, VN, WO, OUT)
    nc.compile()
    return nc


def _emit(nc, tc, XT, FQKT, FVT, RQ, RK, VN, WO, OUT):
    from contextlib import ExitStack

    with ExitStack() as ctx:
        singles = ctx.enter_context(tc.tile_pool(name="singles", bufs=1))
        qkvp = ctx.enter_context(tc.tile_pool(name="qkvp", bufs=1))
        stg_cm = tc.tile_pool(name="stg", bufs=1)
        stg = stg_cm.__enter__()

        # ---- constant / weight loads ----
        fqkt_sb = stg.tile([128, NDCH, N_FQK], F32R, name="fqkt_sb")
        nc.sync.dma_start(out=fqkt_sb, in_=FQKT.ap().rearrange("(c p) j -> p c j", p=128))
        fvt_sb = stg.tile([128, NDCH, N_FV], F32R, name="fvt_sb")
        nc.sync.dma_start(out=fvt_sb, in_=FVT.ap().rearrange("(c p) j -> p c j", p=128))
        rq_sb = stg.tile([N_FQK, GD], F32R, name="rq_sb")
        nc.sync.dma_start(out=rq_sb, in_=RQ.ap())
        rk_sb = stg.tile([N_FQK, GD], F32R, name="rk_sb")
        nc.sync.dma_start(out=rk_sb, in_=RK.ap())
        vn_sb = stg.tile([N_FV, GD], F32R, name="vn_sb")
        nc.sync.dma_start(out=vn_sb, in_=VN.ap())
        wo_sb = singles.tile([128, 4, D], F32R, name="wo_sb")
        nc.sync.dma_start(out=wo_sb, in_=WO.ap().rearrange("(t p) i -> p t i", p=128))

        # f32 scratch constants (memset on float32r fails the ISA check, so
        # constants are built in f32 and copied into f32r tiles by DVE).
        ones_f = singles.tile([128, 128], F32, name="ones_f")
        nc.vector.memset(ones_f, 1.0)
        zeros_sb = singles.tile([128, 1024], F32, name="zeros_sb")
        nc.vector.memset(zeros_sb, 0.0)

        # causal masks for the 4 diagonal-chunk offsets, replicated per head:
        # mask2[k, t, h, q] = 1.0 if q >= k + 128*t else 0.0
        mask2f = stg.tile([128, 4, 2, 512], F32, name="mask2f")
        nc.vector.memset(mask2f, 1.0)
        for t in range(4):
            for h in range(2):
                nc.gpsimd.affine_select(
                    out=mask2f[:, t, h, :],
                    in_=mask2f[:, t, h, :],
                    compare_op=mybir.AluOpType.is_ge,
                    fill=0.0,
                    base=-128 * t,
                    pattern=[[1, 512]],
                    channel_multiplier=-1,
                )
        mask2 = singles.tile([128, 4, 2, 512], F32R, name="mask2")
        nc.vector.tensor_copy(mask2[:, :, :, :], mask2f[:, :, :, :])

        # ---- stage 1: hT = FT.T @ xT  (contraction over D in 8 chunks) ----
        hqkT = stg.tile([N_FQK, S], F32R, name="hqkT")
        hvT = stg.tile([N_FV, S], F32R, name="hvT")
        with tc.tile_pool(name="ph", bufs=1, space="PSUM") as ph:
            hqk_ps = [ph.tile([N_FQK, 512], F32, tag=f"hqk{n}", name=f"hqk_ps{n}") for n in range(4)]
            hv_ps = [ph.tile([N_FV, 512], F32, tag=f"hv{n}", name=f"hv_ps{n}") for n in range(4)]
            for c in range(NDCH):
                xt_c = stg.tile([128, S], F32R, tag="xt", bufs=3, name=f"xt{c}")
                nc.sync.dma_start(out=xt_c, in_=XT.ap()[128 * c:128 * (c + 1), :])
                for n in range(4):
                    nc.tensor.matmul(
                        hqk_ps[n][:, :], fqkt_sb[:, c, :], xt_c[:, 512 * n:512 * (n + 1)],
                        start=(c == 0), stop=(c == NDCH - 1),
                    )
                for n in range(4):
                    nc.tensor.matmul(
                        hv_ps[n][:, :], fvt_sb[:, c, :], xt_c[:, 512 * n:512 * (n + 1)],
                        start=(c == 0), stop=(c == NDCH - 1),
                    )
            for n in range(4):
                nc.vector.tensor_copy(hqkT[:, 512 * n:512 * (n + 1)], hqk_ps[n][:, :])
                nc.vector.tensor_copy(hvT[:, 512 * n:512 * (n + 1)], hv_ps[n][:, :])

        # ---- stage 2: QT, KT (dim-major) and V (s-major) ----
        qt_sb = [qkvp.tile([128, S], F32R, name=f"qt{t}") for t in range(4)]
        kt_sb = [qkvp.tile([128, S], F32R, name=f"kt{t}") for t in range(4)]
        v_sb = qkvp.tile([128, NKCH, HPG, DH + 1], F32R, name="v_sb")
        with tc.tile_pool(name="pqkv", bufs=3, space="PSUM") as pqkv:
            for t in range(4):
                for n in range(4):
                    q_ps = pqkv.tile([128, 512], F32, tag="qkv", name=f"q_ps{t}_{n}")
                    nc.tensor.matmul(
                        q_ps[:, :], rq_sb[:, 128 * t:128 * (t + 1)],
                        hqkT[:, 512 * n:512 * (n + 1)], start=True, stop=True,
                    )
                    nc.vector.tensor_copy(qt_sb[t][:, 512 * n:512 * (n + 1)], q_ps[:, :])
                    k_ps = pqkv.tile([128, 512], F32, tag="qkv", name=f"k_ps{t}_{n}")
                    nc.tensor.matmul(
                        k_ps[:, :], rk_sb[:, 128 * t:128 * (t + 1)],
                        hqkT[:, 512 * n:512 * (n + 1)], start=True, stop=True,
                    )
                    nc.vector.tensor_copy(kt_sb[t][:, 512 * n:512 * (n + 1)], k_ps[:, :])
            for sc in range(NKCH):
                v_ps = pqkv.tile([128, 512], F32, tag="qkv", name=f"v_ps{sc}")
                nc.tensor.matmul(
                    v_ps[:, :], hvT[:, 128 * sc:128 * (sc + 1)], vn_sb[:, :],
                    start=True, stop=True,
                )
                nc.vector.tensor_copy(
                    v_sb[:, sc, :, 0:DH],
                    v_ps.rearrange("p (h d) -> p h d", h=HPG),
                )
            # ones column at dh=64 of every (s-chunk, head): the AV matmul's
            # 65th output row becomes the softmax denominator.
            nc.vector.tensor_copy(
                v_sb[:, :, :, DH:DH + 1],
                ones_f.rearrange("p (a b c) -> p a b c", a=NKCH, b=HPG),
            )

        # ---- stage 3+4: causal attention (scoresT layout) + W_O ----
        stg_cm.__exit__(None, None, None)  # free stage-1/2 SBUF
        aop = ctx.enter_context(tc.tile_pool(name="aop", bufs=1))
        att_outT = [aop.tile([128, S], F32R, name=f"aot{t}") for t in range(4)]
        attp = ctx.enter_context(tc.tile_pool(name="attp", bufs=3))
        nrmp = ctx.enter_context(tc.tile_pool(name="nrmp", bufs=1))
        outp = ctx.enter_context(tc.tile_pool(name="outp", bufs=2))
        scp = ctx.enter_context(tc.tile_pool(name="scp", bufs=2, space="PSUM"))
        avp = ctx.enter_context(tc.tile_pool(name="avp", bufs=1, space="PSUM"))
        wop = ctx.enter_context(tc.tile_pool(name="wop", bufs=1, space="PSUM"))

        for j in range(NQT):
            qs = slice(512 * j, 512 * (j + 1))
            for hp in range(4):
                qt_t, kt_t = qt_sb[hp], kt_sb[hp]
                # one [65, 512] PSUM accumulator per head: rows 0:64 are the
                # unnormalized att_out.T, row 64 (from V's ones column) is the
                # softmax denominator. f32r matmuls require dst base
                # partition 0, so the two heads use separate banks.
                av_a = avp.tile([65, 512], F32, tag="ava", name=f"ava{j}_{hp}")
                av_b = avp.tile([65, 512], F32, tag="avb", name=f"avb{j}_{hp}")
                nkc = 4 * j + 4
                prev = None

                def emit_av(kc, attn_sb, first, last):
                    nc.tensor.matmul(
                        av_a[:, :], v_sb[:, kc, 2 * hp, :], attn_sb[:, 0:512],
                        start=first, stop=last,
                    )
                    nc.tensor.matmul(
                        av_b[:, :], v_sb[:, kc, 2 * hp + 1, :], attn_sb[:, 512:1024],
                        start=first, stop=last,
                    )

                for kc in range(nkc):
                    sc_ps = scp.tile([128, 1024], F32, tag="sc", name=f"sc{j}_{hp}_{kc}")
                    nc.tensor.matmul(
                        sc_ps[:, 0:512], kt_t[0:64, 128 * kc:128 * (kc + 1)],
                        qt_t[0:64, qs], start=True, stop=True, tile_position=(0, 0),
                    )
                    nc.tensor.matmul(
                        sc_ps[:, 512:1024], kt_t[64:128, 128 * kc:128 * (kc + 1)],
                        qt_t[64:128, qs], start=True, stop=True, tile_position=(64, 0),
                    )
                    attn_sb = attp.tile([128, 1024], F32R, tag="at", name=f"at{j}_{hp}_{kc}")
                    t = kc - 4 * j
                    if t < 0:
                        nc.scalar.activation(
                            attn_sb[:, :], sc_ps[:, :],
                            mybir.ActivationFunctionType.Exp, scale=SCALE,
                        )
                    else:
                        a3 = attn_sb.rearrange("p (h q) -> p h q", h=2)
                        s3 = sc_ps.rearrange("p (h q) -> p h q", h=2)
                        z3 = zeros_sb.rearrange("p (h q) -> p h q", h=2)
                        if t > 0:
                            nc.vector.tensor_copy(a3[:, :, 0:128 * t], z3[:, :, 0:128 * t])
                        nc.scalar.activation(
                            a3[:, :, 128 * t:], s3[:, :, 128 * t:],
                            mybir.ActivationFunctionType.Exp, scale=SCALE,
                        )
                        nc.vector.tensor_tensor(
                            out=a3[:, :, 128 * t:], in0=a3[:, :, 128 * t:],
                            in1=mask2[:, t, :, 128 * t:], op=mybir.AluOpType.mult,
                        )
                    if prev is not None:
                        emit_av(prev[0], prev[1], prev[0] == 0, False)
                    prev = (kc, attn_sb)
                emit_av(prev[0], prev[1], prev[0] == 0, True)

                # normalize: recip of the denominator rows (partition 64),
                # partition-broadcast to 64 lanes, one multiply per head.
                # Head a lands directly in att_outT rows 0:64; head b is
                # staged at partitions 0:64 and DMA-moved to rows 64:128
                # (engines cannot shift partitions; DMA can).
                recip_a = nrmp.tile([65, 512], F32, tag="ra", name=f"ra{j}_{hp}")
                nc.vector.reciprocal(out=recip_a[64:65, :], in_=av_a[64:65, :])
                recip_b = nrmp.tile([65, 512], F32, tag="rb", name=f"rb{j}_{hp}")
                nc.vector.reciprocal(out=recip_b[64:65, :], in_=av_b[64:65, :])
                # HW partition_broadcast only reads partition 0 — DMA the
                # reciprocal rows from partition 64 down to partition 0 first.
                r0a = nrmp.tile([1, 512], F32, tag="r0a", name=f"r0a{j}_{hp}")
                nc.sync.dma_start(out=r0a[0:1, :], in_=recip_a[64:65, :])
                r0b = nrmp.tile([1, 512], F32, tag="r0b", name=f"r0b{j}_{hp}")
                nc.sync.dma_start(out=r0b[0:1, :], in_=recip_b[64:65, :])
                bc_a = nrmp.tile([64, 512], F32, tag="bca", name=f"bca{j}_{hp}")
                nc.gpsimd.partition_broadcast(out_ap=bc_a[:, :], in_ap=r0a[0:1, :], channels=64)
                bc_b = nrmp.tile([64, 512], F32, tag="bcb", name=f"bcb{j}_{hp}")
                nc.gpsimd.partition_broadcast(out_ap=bc_b[:, :], in_ap=r0b[0:1, :], channels=64)
                nc.vector.tensor_tensor(
                    out=att_outT[hp][0:64, qs], in0=av_a[0:64, :], in1=bc_a[:, :],
                    op=mybir.AluOpType.mult,
                )
                att_b = nrmp.tile([64, 512], F32R, tag="atb", name=f"atb{j}_{hp}")
                nc.vector.tensor_tensor(
                    out=att_b[:, :], in0=av_b[0:64, :], in1=bc_b[:, :],
                    op=mybir.AluOpType.mult,
                )
                nc.sync.dma_start(out=att_outT[hp][64:128, qs], in_=att_b[:, :])

            # W_O for this q-tile (all head pairs now done for q in qs)
            for ic in range(8):
                wo_ps = wop.tile([128, 512], F32, tag="wo", name=f"wo{j}_{ic}")
                for t in range(4):
                    nc.tensor.matmul(
                        wo_ps[:, :], wo_sb[:, t, 128 * ic:128 * (ic + 1)],
                        att_outT[t][:, qs], start=(t == 0), stop=(t == 3),
                    )
                ob = outp.tile([128, 512], F32, tag="ob", name=f"ob{j}_{ic}")
                nc.vector.tensor_copy(ob[:, :], wo_ps[:, :])
                nc.sync.dma_start(out=OUT.ap()[128 * ic:128 * (ic + 1), qs], in_=ob[:, :])


def _make_in_maps(inputs):
    x = np.asarray(inputs["x"], dtype=np.float32)
    idx_qk = np.asarray(inputs["idx_qk"])
    idx_v = np.asarray(inputs["idx_v"])
    idx_q = np.asarray(inputs["idx_q"])
    idx_k = np.asarray(inputs["idx_k"])
    idx_v2 = np.asarray(inputs["idx_v2"])
    fqk = np.asarray(inputs["feature_qk_neurons"], dtype=np.float32)
    fv = np.asarray(inputs["feature_v_neurons"], dtype=np.float32)
    rq = np.asarray(inputs["relational_q_neurons"], dtype=np.float32)
    rk = np.asarray(inputs["relational_k_neurons"], dtype=np.float32)
    vn = np.asarray(inputs["value_neurons"], dtype=np.float32)
    wo = np.asarray(inputs["W_O"], dtype=np.float32)

    in_maps = []
    for b in range(B):
        xt_b = np.ascontiguousarray(x[b].T)
        fqkt_b = np.ascontiguousarray(fqk[idx_qk[b]].T)
        fvt_b = np.ascontiguousarray(fv[idx_v[b]].T)
        rq_b = rq[idx_q[b]]
        rk_b = rk[idx_k[b]]
        vn_b = vn[idx_v2[b]]
        for g in range(NG):
            gs = slice(512 * g, 512 * (g + 1))
            in_maps.append({
                "xt": xt_b,
                "fqkt": fqkt_b,
                "fvt": fvt_b,
                "rq": np.ascontiguousarray(rq_b[:, gs]),
                "rk": np.ascontiguousarray(rk_b[:, gs]),
                "vn": np.ascontiguousarray(vn_b[:, gs]),
                "wo": np.ascontiguousarray(wo[:, gs].T),
            })
    return in_maps


_cached_nc = None


def _get_nc():
    global _cached_nc
    if _cached_nc is None:
        _cached_nc = _build_program()
    return _cached_nc


def run(inputs, trace=False):
    """Run on 8 NeuronCores; returns (output [B,S,D] f32, BassKernelResults)."""
    from concourse.bass_utils import run_bass_kernel_spmd

    nc = _get_nc()
    in_maps = _make_in_maps(inputs)
    res = run_bass_kernel_spmd(nc, in_maps, core_ids=list(range(2 * B)), trace=trace)
    out = np.empty((B, S, D), dtype=np.float32)
    for b in range(B):
        acc = res.results[2 * b]["out"] + res.results[2 * b + 1]["out"]
        out[b] = acc.T
    return out, res


def kernel(**inputs) -> np.ndarray:
    return run(inputs, trace=False)[0]


# revision 17
# speedup vs baseline: 1.0891x; 1.0891x over previous
"""Trainium2 Bass kernel for nn_NeuronCircuit_86784109183091 (moe_routing).

Reference computation (per batch b):
    h_qk = x[b] @ Fqk[idx_qk[b]].T            [S, 64]
    h_v  = x[b] @ Fv[idx_v[b]].T              [S, 32]
    Q    = h_qk @ Rq[idx_q[b]]                [S, D]
    K    = h_qk @ Rk[idx_k[b]]                [S, D]
    V    = h_v  @ Vn[idx_v2[b]]               [S, D]
    out  = causal_mha(Q, K, V; H=16, dh=64) @ W_O.T

Sharding: 8 cores = 4 batches x 2 head-groups (8 heads each). Gathers and
W_O column slicing happen host-side (index tensors are tiny); each core runs
a dense kernel and returns a partial transposed output [D, S]; the host sums
the two head-group partials per batch and transposes.

On-device layout (per core):
    xT   [D, S]    h computed transposed:  hT = FT.T @ xT        [64/32, S]
    QT/KT [512,S]  as 4 tiles [128, S]; tile t holds heads 2t / 2t+1
    V    [S, 512]  s-major, as [128, 16*8*64] (s-chunk, head, dh)
    scoresT[k, q]  per 128-k-chunk, 512-q-tile; exp on ScalarE; causal via
                   block skipping + precomputed 0/1 masks on diagonal chunks
    att_outT       unnormalized via col-packed AV matmuls; denominators via
                   ones-vector matmuls; normalized with reciprocal +
                   gpsimd partition_broadcast + one DVE multiply
    out.T += WO_g slice.T @ att_outT  (accumulated over 4 dim-chunks)

All matmuls run in float32r (full PE rate at N>=512; ~1e-4 matmul rel err).
"""

import sys

for _p in ("/opt/trn_rl_repo",):
    if _p not in sys.path:
        sys.path.append(_p)

import numpy as np

import concourse.bacc as bacc
import concourse.mybir as mybir
import concourse.tile as tile

F32 = mybir.dt.float32
F32R = mybir.dt.float32r

B = 4
S = 2048
D = 1024
NG = 2          # head groups (cores per batch)
HPG = 8         # heads per group
DH = 64
GD = NG and 512  # dims per group
N_FQK = 64      # gathered feature_qk rows
N_FV = 32       # gathered feature_v rows
SCALE = 1.0 / 8.0

NKCH = S // 128   # 16 k-chunks of 128
NQT = S // 512    # 4 q-tiles of 512
NDCH = D // 128   # 8 d-chunks of 128


def _build_program():
    nc = bacc.Bacc("TRN2", target_bir_lowering=False, debug=False)

    XT = nc.dram_tensor("xt", [D, S], F32R, kind="ExternalInput")
    FQKT = nc.dram_tensor("fqkt", [D, N_FQK], F32R, kind="ExternalInput")
    FVT = nc.dram_tensor("fvt", [D, N_FV], F32R, kind="ExternalInput")
    RQ = nc.dram_tensor("rq", [N_FQK, GD], F32R, kind="ExternalInput")
    RK = nc.dram_tensor("rk", [N_FQK, GD], F32R, kind="ExternalInput")
    VN = nc.dram_tensor("vn", [N_FV, GD], F32R, kind="ExternalInput")
    WO = nc.dram_tensor("wo", [GD, D], F32R, kind="ExternalInput")
    OUT = nc.dram_tensor("out", [D, S], F32, kind="ExternalOutput")

    with tile.TileContext(nc) as tc:
        _emit(nc, tc, XT, FQKT, FVT, RQ, RK, VN, WO, OUT)
    nc.compile()
    return nc


def _emit(nc, tc, XT, FQKT, FVT, RQ, RK, VN, WO, OUT):
    from contextlib import ExitStack

    with ExitStack() as ctx:
        singles = ctx.enter_context(tc.tile_pool(name="singles", bufs=1))
        qkvp = ctx.enter_context(tc.tile_pool(name="qkvp", bufs=1))
        stg_cm = tc.tile_pool(name="stg", bufs=1)
        stg = stg_cm.__enter__()

        # ---- constant / weight loads ----
        fqkt_sb = stg.tile([128, NDCH, N_FQK], F32R, name="fqkt_sb")
        nc.sync.dma_start(out=fqkt_sb, in_=FQKT.ap().rearrange("(c p) j -> p c j", p=128))
        fvt_sb = stg.tile([128, NDCH, N_FV], F32R, name="fvt_sb")
        nc.sync.dma_start(out=fvt_sb, in_=FVT.ap().rearrange("(c p) j -> p c j", p=128))
        rq_sb = stg.tile([N_FQK, GD], F32R, name="rq_sb")
        nc.sync.dma_start(out=rq_sb, in_=RQ.ap())
        rk_sb = stg.tile([N_FQK, GD], F32R, name="rk_sb")
        nc.sync.dma_start(out=rk_sb, in_=RK.ap())
        vn_sb = stg.tile([N_FV, GD], F32R, name="vn_sb")
        nc.sync.dma_start(out=vn_sb, in_=VN.ap())
        wo_sb = singles.tile([128, 4, D], F32R, name="wo_sb")
        nc.sync.dma_start(out=wo_sb, in_=WO.ap().rearrange("(t p) i -> p t i", p=128))

        # f32 scratch constants (memset on float32r fails the ISA check, so
        # constants are built in f32 and copied into f32r tiles by DVE).
        ones_f = singles.tile([128, 128], F32, name="ones_f")
        nc.vector.memset(ones_f, 1.0)

        # ---- stage 1: hT = FT.T @ xT  (contraction over D in 8 chunks) ----
        hqkT = stg.tile([N_FQK, S], F32R, name="hqkT")
        hvT = stg.tile([N_FV, S], F32R, name="hvT")
        with tc.tile_pool(name="ph", bufs=1, space="PSUM") as ph:
            hqk_ps = [ph.tile([N_FQK, 512], F32, tag=f"hqk{n}", name=f"hqk_ps{n}") for n in range(4)]
            hv_ps = [ph.tile([N_FV, 512], F32, tag=f"hv{n}", name=f"hv_ps{n}") for n in range(4)]
            for c in range(NDCH):
                xt_c = stg.tile([128, S], F32R, tag="xt", bufs=3, name=f"xt{c}")
                nc.sync.dma_start(out=xt_c, in_=XT.ap()[128 * c:128 * (c + 1), :])
                for n in range(4):
                    nc.tensor.matmul(
                        hqk_ps[n][:, :], fqkt_sb[:, c, :], xt_c[:, 512 * n:512 * (n + 1)],
                        start=(c == 0), stop=(c == NDCH - 1),
                    )
                for n in range(4):
                    nc.tensor.matmul(
                        hv_ps[n][:, :], fvt_sb[:, c, :], xt_c[:, 512 * n:512 * (n + 1)],
                        start=(c == 0), stop=(c == NDCH - 1),
                    )
            for n in range(4):
                nc.vector.tensor_copy(hqkT[:, 512 * n:512 * (n + 1)], hqk_ps[n][:, :])
                nc.vector.tensor_copy(hvT[:, 512 * n:512 * (n + 1)], hv_ps[n][:, :])

        # ---- stage 2: QT, KT (dim-major) and V (s-major) ----
        qt_sb = [qkvp.tile([128, S], F32R, name=f"qt{t}") for t in range(4)]
        kt_sb = [qkvp.tile([128, S], F32R, name=f"kt{t}") for t in range(4)]
        v_sb = qkvp.tile([128, NKCH, HPG, DH + 1], F32R, name="v_sb")
        with tc.tile_pool(name="pqkv", bufs=3, space="PSUM") as pqkv:
            for t in range(4):
                for n in range(4):
                    q_ps = pqkv.tile([128, 512], F32, tag="qkv", name=f"q_ps{t}_{n}")
                    nc.tensor.matmul(
                        q_ps[:, :], rq_sb[:, 128 * t:128 * (t + 1)],
                        hqkT[:, 512 * n:512 * (n + 1)], start=True, stop=True,
                    )
                    nc.scalar.copy(qt_sb[t][:, 512 * n:512 * (n + 1)], q_ps[:, :])
                    k_ps = pqkv.tile([128, 512], F32, tag="qkv", name=f"k_ps{t}_{n}")
                    nc.tensor.matmul(
                        k_ps[:, :], rk_sb[:, 128 * t:128 * (t + 1)],
                        hqkT[:, 512 * n:512 * (n + 1)], start=True, stop=True,
                    )
                    nc.vector.tensor_copy(kt_sb[t][:, 512 * n:512 * (n + 1)], k_ps[:, :])
            for sc in range(NKCH):
                v_ps = pqkv.tile([128, 512], F32, tag="qkv", name=f"v_ps{sc}")
                nc.tensor.matmul(
                    v_ps[:, :], hvT[:, 128 * sc:128 * (sc + 1)], vn_sb[:, :],
                    start=True, stop=True,
                )
                nc.vector.tensor_copy(
                    v_sb[:, sc, :, 0:DH],
                    v_ps.rearrange("p (h d) -> p h d", h=HPG),
                )
            # ones column at dh=64 of every (s-chunk, head): the AV matmul's
            # 65th output row becomes the softmax denominator.
            nc.vector.tensor_copy(
                v_sb[:, :, :, DH:DH + 1],
                ones_f.rearrange("p (a b c) -> p a b c", a=NKCH, b=HPG),
            )

        # ---- stage 3+4: causal attention (scoresT layout) + W_O ----
        stg_cm.__exit__(None, None, None)  # free stage-1/2 SBUF
        aop = ctx.enter_context(tc.tile_pool(name="aop", bufs=1))
        att_outT = [aop.tile([128, S], F32R, name=f"aot{t}") for t in range(4)]
        attp = ctx.enter_context(tc.tile_pool(name="attp", bufs=4))
        nrmp = ctx.enter_context(tc.tile_pool(name="nrmp", bufs=1))
        outp = ctx.enter_context(tc.tile_pool(name="outp", bufs=2))
        scp = ctx.enter_context(tc.tile_pool(name="scp", bufs=2, space="PSUM"))
        avp = ctx.enter_context(tc.tile_pool(name="avp", bufs=1, space="PSUM"))
        wop = ctx.enter_context(tc.tile_pool(name="wop", bufs=2, space="PSUM"))

        for j in range(NQT):
            qs = slice(512 * j, 512 * (j + 1))
            for hp in range(4):
                qt_t, kt_t = qt_sb[hp], kt_sb[hp]
                # one [65, 512] PSUM accumulator per head: rows 0:64 are the
                # unnormalized att_out.T, row 64 (from V's ones column) is the
                # softmax denominator. f32r matmuls require dst base
                # partition 0, so the two heads use separate banks.
                av_a = avp.tile([65, 512], F32, tag="ava", name=f"ava{j}_{hp}")
                av_b = avp.tile([65, 512], F32, tag="avb", name=f"avb{j}_{hp}")
                nkc = 4 * j + 4
                prev = None

                def emit_av(kc, attn_sb, first, last):
                    nc.tensor.matmul(
                        av_a[:, :], v_sb[:, kc, 2 * hp, :], attn_sb[:, 0:512],
                        start=first, stop=last,
                    )
                    nc.tensor.matmul(
                        av_b[:, :], v_sb[:, kc, 2 * hp + 1, :], attn_sb[:, 512:1024],
                        start=first, stop=last,
                    )

                for kc in range(nkc):
                    sc_ps = scp.tile([128, 1024], F32, tag="sc", name=f"sc{j}_{hp}_{kc}")
                    nc.tensor.matmul(
                        sc_ps[:, 0:512], kt_t[0:64, 128 * kc:128 * (kc + 1)],
                        qt_t[0:64, qs], start=True, stop=True, tile_position=(0, 0),
                    )
                    nc.tensor.matmul(
                        sc_ps[:, 512:1024], kt_t[64:128, 128 * kc:128 * (kc + 1)],
                        qt_t[64:128, qs], start=True, stop=True, tile_position=(64, 0),
                    )
                    attn_sb = attp.tile([128, 1024], F32R, tag="at", name=f"at{j}_{hp}_{kc}")
                    t = kc - 4 * j
                    nc.scalar.activation(
                        attn_sb[:, :], sc_ps[:, :],
                        mybir.ActivationFunctionType.Exp, scale=SCALE,
                    )
                    if t >= 0:
                        # zero the causally-masked region in place:
                        # keep where q - k - 128t >= 0 (q,k local to block)
                        a3 = attn_sb.rearrange("p (h q) -> p h q", h=2)
                        nc.gpsimd.affine_select(
                            out=a3[:, :, :], in_=a3[:, :, :],
                            compare_op=mybir.AluOpType.is_ge,
                            fill=0.0, base=-128 * t,
                            pattern=[[0, 2], [1, 512]],
                            channel_multiplier=-1,
                        )
                    if prev is not None:
                        emit_av(prev[0], prev[1], prev[0] == 0, False)
                    prev = (kc, attn_sb)
                emit_av(prev[0], prev[1], prev[0] == 0, True)

                # normalize: recip of the denominator rows (partition 64),
                # partition-broadcast to 64 lanes, one multiply per head.
                # Head a lands directly in att_outT rows 0:64; head b is
                # staged at partitions 0:64 and DMA-moved to rows 64:128
                # (engines cannot shift partitions; DMA can).
                recip_a = nrmp.tile([65, 512], F32, tag="ra", name=f"ra{j}_{hp}")
                nc.vector.reciprocal(out=recip_a[64:65, :], in_=av_a[64:65, :])
                recip_b = nrmp.tile([65, 512], F32, tag="rb", name=f"rb{j}_{hp}")
                nc.vector.reciprocal(out=recip_b[64:65, :], in_=av_b[64:65, :])
                # HW partition_broadcast only reads partition 0 — DMA the
                # reciprocal rows from partition 64 down to partition 0 first.
                r0a = nrmp.tile([1, 512], F32, tag="r0a", name=f"r0a{j}_{hp}")
                nc.sync.dma_start(out=r0a[0:1, :], in_=recip_a[64:65, :])
                r0b = nrmp.tile([1, 512], F32, tag="r0b", name=f"r0b{j}_{hp}")
                nc.sync.dma_start(out=r0b[0:1, :], in_=recip_b[64:65, :])
                bc_a = nrmp.tile([64, 512], F32, tag="bca", name=f"bca{j}_{hp}")
                nc.gpsimd.partition_broadcast(out_ap=bc_a[:, :], in_ap=r0a[0:1, :], channels=64)
                bc_b = nrmp.tile([64, 512], F32, tag="bcb", name=f"bcb{j}_{hp}")
                nc.gpsimd.partition_broadcast(out_ap=bc_b[:, :], in_ap=r0b[0:1, :], channels=64)
                nc.vector.tensor_tensor(
                    out=att_outT[hp][0:64, qs], in0=av_a[0:64, :], in1=bc_a[:, :],
                    op=mybir.AluOpType.mult,
                )
                att_b = nrmp.tile([64, 512], F32R, tag="atb", name=f"atb{j}_{hp}")
                nc.vector.tensor_tensor(
                    out=att_b[:, :], in0=av_b[0:64, :], in1=bc_b[:, :],
                    op=mybir.AluOpType.mult,
                )
                nc.sync.dma_start(out=att_outT[hp][64:128, qs], in_=att_b[:, :])

            # W_O for this q-tile (all head pairs now done for q in qs)
            for ic in range(8):
                wo_ps = wop.tile([128, 512], F32, tag="wo", name=f"wo{j}_{ic}")
                for t in range(4):
                    nc.tensor.matmul(
                        wo_ps[:, :], wo_sb[:, t, 128 * ic:128 * (ic + 1)],
                        att_outT[t][:, qs], start=(t == 0), stop=(t == 3),
                    )
                ob = outp.tile([128, 512], F32, tag="ob", name=f"ob{j}_{ic}")
                nc.vector.tensor_copy(ob[:, :], wo_ps[:, :])
                nc.sync.dma_start(out=OUT.ap()[128 * ic:128 * (ic + 1), qs], in_=ob[:, :])


def _make_in_maps(inputs):
    x = np.asarray(inputs["x"], dtype=np.float32)
    idx_qk = np.asarray(inputs["idx_qk"])
    idx_v = np.asarray(inputs["idx_v"])
    idx_q = np.asarray(inputs["idx_q"])
    idx_k = np.asarray(inputs["idx_k"])
    idx_v2 = np.asarray(inputs["idx_v2"])
    fqk = np.asarray(inputs["feature_qk_neurons"], dtype=np.float32)
    fv = np.asarray(inputs["feature_v_neurons"], dtype=np.float32)
    rq = np.asarray(inputs["relational_q_neurons"], dtype=np.float32)
    rk = np.asarray(inputs["relational_k_neurons"], dtype=np.float32)
    vn = np.asarray(inputs["value_neurons"], dtype=np.float32)
    wo = np.asarray(inputs["W_O"], dtype=np.float32)

    in_maps = []
    for b in range(B):
        xt_b = np.ascontiguousarray(x[b].T)
        fqkt_b = np.ascontiguousarray(fqk[idx_qk[b]].T)
        fvt_b = np.ascontiguousarray(fv[idx_v[b]].T)
        rq_b = rq[idx_q[b]]
        rk_b = rk[idx_k[b]]
        vn_b = vn[idx_v2[b]]
        for g in range(NG):
            gs = slice(512 * g, 512 * (g + 1))
            in_maps.append({
                "xt": xt_b,
                "fqkt": fqkt_b,
                "fvt": fvt_b,
                "rq": np.ascontiguousarray(rq_b[:, gs]),
                "rk": np.ascontiguousarray(rk_b[:, gs]),
                "vn": np.ascontiguousarray(vn_b[:, gs]),
                "wo": np.ascontiguousarray(wo[:, gs].T),
            })
    return in_maps


_cached_nc = None


def _get_nc():
    global _cached_nc
    if _cached_nc is None:
        _cached_nc = _build_program()
    return _cached_nc


def run(inputs, trace=False):
    """Run on 8 NeuronCores; returns (output [B,S,D] f32, BassKernelResults)."""
    from concourse.bass_utils import run_bass_kernel_spmd

    nc = _get_nc()
    in_maps = _make_in_maps(inputs)
    res = run_bass_kernel_spmd(nc, in_maps, core_ids=list(range(2 * B)), trace=trace)
    out = np.empty((B, S, D), dtype=np.float32)
    for b in range(B):
        acc = res.results[2 * b]["out"] + res.results[2 * b + 1]["out"]
        out[b] = acc.T
    return out, res


def kernel(**inputs) -> np.ndarray:
    return run(inputs, trace=False)[0]
